# revision 21
# baseline (speedup 1.0000x reference)
"""Trainium2 Bass kernel for nn_DelayedMLP (B=32, S=2048, I=256, H=512, O=256).

Strategy
--------
Sequence-parallel decomposition of the recurrent scan: the buffer state's
dependence on the past decays geometrically (|d buf_t / d buf_{t-w}| ~ 0.5^w),
so a chain started from buf=0 a few steps early converges to the true state to
fp32 precision.  Each of the 8 cores takes a 256-step S-chunk; within a core
the chunk is split into 16 chains of 16 steps, each warmed up for 16 steps.
All 16 chains advance in lockstep, vectorized with the batch (16*32 = 512
tokens per device step), which makes the per-step gate matmul a full-width
[128,128] x [128,256] operation instead of a [4,256] sliver.

Algebraic simplifications used:
  e_t   = x_t * sigmoid(-(x_t@Wg + bg))          (input gate, bulk-precomputable)
  u_t   = buf_{t-1} + e_t
  buf_t = u_t * sigmoid(-(u_t@Wg + bg))
  out_t = x_t + buf_{t-1} - buf_t                 (imm + release telescopes)

The pointwise MLP runs fused in the same pass over each step's 512 tokens.
All matmul operands are fp16 (same 10-bit mantissa as TF32/fp32r, but 16-bit
dtypes get fast background weight loads and 2x DVE throughput); accumulation
is fp32 in PSUM.  Host-side work is layout only: gather/pad/transpose shards,
un-transpose the result.
"""

import numpy as np
from contextlib import ExitStack

import concourse.bass as bass
import concourse.bacc as bacc
import concourse.tile as tile
from concourse import mybir
from concourse.bass_utils import run_bass_kernel_spmd

F32 = mybir.dt.float32
F32R = mybir.dt.float32r
F16 = mybir.dt.float16

B, S, I, H, O = 32, 2048, 256, 512, 256
NCORES = 8
CHUNK = S // NCORES          # 256 timesteps per core
NCHAIN = 16                  # chains per core
CLEN = CHUNK // NCHAIN       # 16 chunk steps per chain
WARM = 16                    # warmup steps per chain
LSTEP = WARM + CLEN          # 32 device steps
TOK = NCHAIN * B             # 512 tokens per device step
FREE = 2 * TOK               # 1024 = two I-chunk segments


def build_kernel():
    nc = bacc.Bacc("TRN2", target_bir_lowering=False, debug=False)

    xT = nc.dram_tensor("xT", [128, LSTEP, FREE], F16, kind="ExternalInput").ap()
    wg_d = nc.dram_tensor("Wg", [I, I], F16, kind="ExternalInput").ap()
    w1_d = nc.dram_tensor("W1", [I, H], F16, kind="ExternalInput").ap()
    w2_d = nc.dram_tensor("W2", [H, H], F16, kind="ExternalInput").ap()
    w3_d = nc.dram_tensor("W3", [H, O], F16, kind="ExternalInput").ap()
    nbg_d = nc.dram_tensor("nbg", [2, 128, 1], F32, kind="ExternalInput").ap()
    b1_d = nc.dram_tensor("b1c", [4, 128, 1], F32, kind="ExternalInput").ap()
    b2_d = nc.dram_tensor("b2c", [4, 128, 1], F32, kind="ExternalInput").ap()
    b3_d = nc.dram_tensor("b3c", [2, 128, 1], F32, kind="ExternalInput").ap()
    outT = nc.dram_tensor("outT", [128, CLEN, FREE], F32, kind="ExternalOutput").ap()

    SIG = mybir.ActivationFunctionType.Sigmoid
    RELU = mybir.ActivationFunctionType.Relu
    ADD = mybir.AluOpType.add
    MAX = mybir.AluOpType.max

    with tile.TileContext(nc) as tc, ExitStack() as ctx:
        wpool = ctx.enter_context(tc.tile_pool(name="weights", bufs=1))
        xpool = ctx.enter_context(tc.tile_pool(name="xt", bufs=5))
        sdpool = ctx.enter_context(tc.tile_pool(name="sd", bufs=4))
        epool = ctx.enter_context(tc.tile_pool(name="e", bufs=5))
        upool = ctx.enter_context(tc.tile_pool(name="u", bufs=3))
        spool = ctx.enter_context(tc.tile_pool(name="s", bufs=3))
        bpool = ctx.enter_context(tc.tile_pool(name="buf", bufs=4))
        cpool = ctx.enter_context(tc.tile_pool(name="c", bufs=2))
        ctpool = ctx.enter_context(tc.tile_pool(name="ctmp", bufs=2))
        h1pool = ctx.enter_context(tc.tile_pool(name="h1", bufs=4))
        h2pool = ctx.enter_context(tc.tile_pool(name="h2", bufs=3))
        opool = ctx.enter_context(tc.tile_pool(name="osb", bufs=2))
        pd = ctx.enter_context(tc.tile_pool(name="pd", bufs=3, space="PSUM"))
        pz = ctx.enter_context(tc.tile_pool(name="pz", bufs=2, space="PSUM"))
        pm = ctx.enter_context(tc.tile_pool(name="pm", bufs=3, space="PSUM"))

        # --- resident weights: one DMA per matrix, sliced into lhsT blocks ---
        def load_blocks(src, kk, cols, name):
            t = wpool.tile([128, kk * cols], F16, tag=name, name=name)
            nc.sync.dma_start(
                t[:].rearrange("p (k c) -> p k c", k=kk),
                src.rearrange("(k p) c -> p k c", p=128),
            )
            return {
                (k, m): t[:, k * cols + m * 128:k * cols + (m + 1) * 128]
                for k in range(kk)
                for m in range(cols // 128)
            }

        wg = load_blocks(wg_d, 2, I, "wgt")

        def load_bias(src, n, name):
            t = wpool.tile([128, n], F32, tag=name, name=name)
            nc.sync.dma_start(
                t[:].rearrange("p (m one) -> p m one", one=1),
                src.rearrange("m p one -> p m one"),
            )
            return [t[:, m:m + 1] for m in range(n)]

        nbg = load_bias(nbg_d, 2, "nbgt")

        def emit_gate(t):
            """Load x^T(t) and compute the input gate + e(t).  Warmup-phase
            gate psums borrow the MLP pool (idle until the chunk phase)."""
            xt = xpool.tile([128, FREE], F16, tag="xt", name=f"xt{t}")
            nc.sync.dma_start(xt[:], xT[:, t, :])
            gp = (pm if t % 2 else pd) if t < WARM else pd
            zd = [gp.tile([128, TOK], F32, tag=gp is pm and "pm" or "pd", name=f"zd{t}_{i}") for i in range(2)]
            for m in range(2):
                for k in range(2):
                    nc.tensor.matmul(
                        zd[m][:],
                        wg[(k, m)],
                        xt[:, k * TOK:(k + 1) * TOK],
                        start=(k == 0),
                        stop=(k == 1),
                    )
            sd = sdpool.tile([128, FREE], F16, tag="sd", name=f"sd{t}")
            for m in range(2):
                nc.scalar.activation(
                    sd[:, m * TOK:(m + 1) * TOK], zd[m][:], SIG,
                    bias=nbg[m], scale=-1.0,
                )
            e = epool.tile([128, FREE], F16, tag="e", name=f"e{t}")
            for h in range(2):
                sl = slice(h * TOK, (h + 1) * TOK)
                nc.vector.tensor_mul(e[:, sl], xt[:, sl], sd[:, sl])
            return xt, e

        buf_prev = None
        gates = [emit_gate(0), emit_gate(1)]
        w1 = load_blocks(w1_d, 2, H, "w1t")
        w2 = load_blocks(w2_d, 4, H, "w2t")
        w3 = load_blocks(w3_d, 4, O, "w3t")
        b1c = load_bias(b1_d, 4, "b1t")
        b2c = load_bias(b2_d, 4, "b2t")
        b3c = load_bias(b3_d, 2, "b3t")
        for t in range(LSTEP):
            xt, e = gates[t]
            if t + 2 < LSTEP:
                gates.append(emit_gate(t + 2))

            # --- state update ------------------------------------------------
            u = upool.tile([128, FREE], F16, tag="u", name=f"u{t}")
            for h in range(2):
                sl = slice(h * TOK, (h + 1) * TOK)
                if t == 0:
                    nc.vector.tensor_scalar_add(u[:, sl], e[:, sl], 0.0)
                else:
                    nc.vector.tensor_add(u[:, sl], buf_prev[:, sl], e[:, sl])

            zz = [pz.tile([128, TOK], F32, tag="pz", name=f"zz{t}_{i}") for i in range(2)]
            for m in range(2):
                for k in range(2):
                    nc.tensor.matmul(
                        zz[m][:],
                        wg[(k, m)],
                        u[:, k * TOK:(k + 1) * TOK],
                        start=(k == 0),
                        stop=(k == 1),
                    )
            s = spool.tile([128, FREE], F16, tag="s", name=f"s{t}")
            for m in range(2):
                nc.scalar.activation(
                    s[:, m * TOK:(m + 1) * TOK], zz[m][:], SIG,
                    bias=nbg[m], scale=-1.0,
                )
            buf = bpool.tile([128, FREE], F16, tag="buf", name=f"buf{t}")
            for h in range(2):
                sl = slice(h * TOK, (h + 1) * TOK)
                nc.vector.tensor_mul(buf[:, sl], u[:, sl], s[:, sl])

            if t >= WARM:
                # --- combined output: c = x + buf_prev - buf -----------------
                ct = ctpool.tile([128, FREE], F16, tag="ctmp", name=f"ct{t}")
                c = cpool.tile([128, FREE], F16, tag="c", name=f"c{t}")
                for h in range(2):
                    sl = slice(h * TOK, (h + 1) * TOK)
                    nc.gpsimd.tensor_sub(ct[:, sl], buf_prev[:, sl], buf[:, sl])
                    nc.vector.tensor_add(c[:, sl], ct[:, sl], xt[:, sl])

                # --- MLP layer 1: h1 = relu(c @ W1 + b1) ---------------------
                h1 = h1pool.tile([128, 4 * TOK], F16, tag="h1", name=f"h1_{t}")
                for m in range(4):
                    ph = pm.tile([128, TOK], F32, tag="pm", name=f"p1_{t}_{m}")
                    for k in range(2):
                        nc.tensor.matmul(
                            ph[:],
                            w1[(k, m)],
                            c[:, k * TOK:(k + 1) * TOK],
                            start=(k == 0),
                            stop=(k == 1),
                        )
                    nc.scalar.activation(
                        h1[:, m * TOK:(m + 1) * TOK], ph[:], RELU, bias=b1c[m]
                    )

                # --- MLP layer 2: h2 = relu(h1 @ W2 + b2) --------------------
                h2 = h2pool.tile([128, 4 * TOK], F16, tag="h2", name=f"h2_{t}")
                for m in range(4):
                    ph = pm.tile([128, TOK], F32, tag="pm", name=f"p2_{t}_{m}")
                    for k in range(4):
                        nc.tensor.matmul(
                            ph[:],
                            w2[(k, m)],
                            h1[:, k * TOK:(k + 1) * TOK],
                            start=(k == 0),
                            stop=(k == 3),
                        )
                    nc.vector.tensor_scalar(
                        h2[:, m * TOK:(m + 1) * TOK], ph[:],
                        b2c[m], 0.0, op0=ADD, op1=MAX,
                    )

                # --- MLP layer 3: o = h2 @ W3 + b3 ---------------------------
                osb = opool.tile([128, FREE], F32, tag="osb", name=f"osb{t}")
                for m in range(2):
                    ph = pm.tile([128, TOK], F32, tag="pm", name=f"p3_{t}_{m}")
                    for k in range(4):
                        nc.tensor.matmul(
                            ph[:],
                            w3[(k, m)],
                            h2[:, k * TOK:(k + 1) * TOK],
                            start=(k == 0),
                            stop=(k == 3),
                        )
                    nc.vector.tensor_scalar_add(
                        osb[:, m * TOK:(m + 1) * TOK], ph[:], b3c[m]
                    )
                    nc.sync.dma_start(
                        outT[:, t - WARM, m * TOK:(m + 1) * TOK],
                        osb[:, m * TOK:(m + 1) * TOK],
                    )

            buf_prev = buf

    nc.compile()
    return nc


def shard_inputs(x, Wg, bg, W1, b1, W2, b2, W3, b3):
    """Pure layout work: build the per-core transposed/gathered input dict."""
    x = np.ascontiguousarray(np.asarray(x, np.float16))
    xp = np.pad(x, ((0, 0), (WARM, 0), (0, 0)))  # [B, WARM+S, I]

    common = {
        "Wg": np.ascontiguousarray(np.asarray(Wg, np.float16)),
        "W1": np.ascontiguousarray(np.asarray(W1, np.float16)),
        "W2": np.ascontiguousarray(np.asarray(W2, np.float16)),
        "W3": np.ascontiguousarray(np.asarray(W3, np.float16)),
        "nbg": np.ascontiguousarray((-np.asarray(bg, np.float32)).reshape(2, 128, 1)),
        "b1c": np.ascontiguousarray(np.asarray(b1, np.float32).reshape(4, 128, 1)),
        "b2c": np.ascontiguousarray(np.asarray(b2, np.float32).reshape(4, 128, 1)),
        "b3c": np.ascontiguousarray(np.asarray(b3, np.float32).reshape(2, 128, 1)),
    }

    in_maps = []
    for k in range(NCORES):
        # window[b, j, t, i] = xp[b, k*CHUNK + j*CLEN + t, i]
        starts = k * CHUNK + np.arange(NCHAIN) * CLEN
        idx = starts[:, None] + np.arange(LSTEP)[None, :]  # [j, t]
        win = xp[:, idx, :]                                # [B, j, t, I]
        win = win.reshape(B, NCHAIN, LSTEP, 2, 128)        # [b, j, t, seg, p]
        xTc = win.transpose(4, 2, 3, 1, 0).reshape(128, LSTEP, FREE)
        in_maps.append({"xT": np.ascontiguousarray(xTc), **common})
    return in_maps


def unshard_output(results):
    out = np.empty((B, S, O), np.float32)
    for k in range(NCORES):
        r_ = results[k]["outT"].reshape(128, CLEN, 2, NCHAIN, B)
        # [p, tc, seg, j, b] -> [b, j, tc, seg, p]
        blk = r_.transpose(4, 3, 1, 2, 0).reshape(B, CHUNK, O)
        out[:, k * CHUNK:(k + 1) * CHUNK, :] = blk
    return out


_NC_CACHE = {}


def _get_nc():
    if "nc" not in _NC_CACHE:
        _NC_CACHE["nc"] = build_kernel()
    return _NC_CACHE["nc"]


def kernel(x, Wg, bg, W1, b1, W2, b2, W3, b3, _trace=False, _trace_kwargs=None):
    nc = _get_nc()
    in_maps = shard_inputs(x, Wg, bg, W1, b1, W2, b2, W3, b3)
    res = run_bass_kernel_spmd(
        nc, in_maps, list(range(NCORES)), trace=_trace,
        **(_trace_kwargs or {}),
    )
    out = unshard_output(res.results)
    if _trace:
        kernel.last_results = res
    return out
